# revision 12
# baseline (speedup 1.0000x reference)
"""Chamfer distance (nn_ChamferDistance) Trainium2 Bass kernel.

Computes, for xyz1/xyz2 of shape (4, 8192, 3) fp32:
    dist[n, m] = |p_n|^2 + |q_m|^2 - 2 p_n.q_m   (per batch)
    dist1 = min over m, dist2 = min over n
Returns (dist1, dist2), each (4, 8192) fp32 — same as the reference.

Strategy:
  - The pairwise-distance matrix is produced directly by the TensorEngine via
    an augmented inner product: u_a . v_b = sq(P)[a] + sq(Q)[b] - 2 P_a.Q_b.
    All factors are split into 3 bf16 planes (hi/lo/lolo) so every product the
    PE forms is exact in fp32; dropped cross terms are ~2^-26 relative.  K=24
    contraction rows, bf16: a [128x512] distance tile costs ~512 PE cycles.
  - Sharding: 8 cores = 4 batches x 2 halves.  Each core runs TWO layouts:
      A: partitions = its half of N, free = all M  -> dist1 rows (min over free)
      B: partitions = its half of M, free = all N  -> dist2 rows (min over free)
    so both outputs are pure free-axis min-reductions; no partition reduce and
    no cross-core combine is needed.
  - Per 128-row tile, matmuls fill PSUM groups of [128, 2048] (4 banks).  The
    row-min of a group is one VectorEngine tensor_scalar with a min-accumulator.
    For ~3 of every 4 groups the ScalarEngine first copies PSUM->SBUF so the
    reduction runs in the DVE's 2x two-port mode; the rest reduce directly from
    PSUM at 1x.  This balances ACT (copies) against DVE (reduces), the two
    engines that can touch PSUM.
"""

import numpy as np
import ml_dtypes

import concourse.bacc as bacc
import concourse.tile as tile
import concourse.mybir as mybir
from concourse import bass_utils

B = 4
N = 8192
M = 8192
NCORES = 8
NSH = N // 2          # rows per core per layout
K = 24                # augmented contraction rows

BF16 = mybir.dt.bfloat16
F32 = mybir.dt.float32
MIN = mybir.AluOpType.min
ADD = mybir.AluOpType.add
X = mybir.AxisListType.X
BIG = 1.0e30


def _emit_layout(tc, pools, lhs_sb, rhs_sb, dst, nt, m, gf):
    """One layout: dst[:, i] = min over free of (lhsT[:, i-tile].T @ rhs)."""
    nc = tc.nc
    ng = m // gf
    nj = gf // 512
    psum_pool, stage_pool, rowm_pool = pools
    for i in range(nt):
        rowm = rowm_pool.tile([128, ng], F32)
        for g in range(ng):
            ps = psum_pool.tile([128, gf], F32, tag="ps")
            for jj in range(nj):
                nc.tensor.matmul(
                    ps[:, jj * 512:(jj + 1) * 512],
                    lhs_sb[:, i * 128:(i + 1) * 128],
                    rhs_sb[:, g * gf + jj * 512: g * gf + (jj + 1) * 512],
                    start=True,
                    stop=True,
                )
            # ~71% of groups go via an ACT copy to SBUF so the min-reduce
            # runs in the DVE 2x mode; the rest reduce from PSUM at 1x.
            nacts = int(0.71 * ng)
            rem = 0.71 * ng - nacts
            act_staged = (g < nacts) or (g == nacts and (i % 4) < rem * 4)
            if act_staged:
                st = stage_pool.tile([128, gf], F32)
                nc.scalar.copy(st[:], ps[:])
                red_src = st
            else:
                red_src = ps
            nc.vector.tensor_scalar(
                red_src[:], red_src[:], 0.0, None, op0=ADD, op1=MIN,
                accum_out=rowm[:, g:g + 1])
        nc.vector.tensor_reduce(dst[:, i:i + 1], rowm[:], axis=X, op=MIN)


def build_body(tc, lhsT_a, rhs_a, lhsT_b, rhs_b, d1t, d2t, nt, m, gf, repeat=1):
    """Emit the kernel body into TileContext `tc`.

    lhsT_a: [K, nt*128] bf16 AP  (augmented rows of this core's N-half)
    rhs_a:  [K, m]      bf16 AP  (augmented rows of all of xyz2)
    lhsT_b: [K, nt*128] bf16 AP  (augmented rows of this core's M-half)
    rhs_b:  [K, m]      bf16 AP  (augmented rows of all of xyz1)
    d1t, d2t: [128, nt] f32 APs out (row r of tile i -> point i*128 + r)
    """
    nc = tc.nc
    with (
        tc.tile_pool(name="inp", bufs=1) as inp_pool,
        tc.tile_pool(name="acc", bufs=1) as acc_pool,
        tc.tile_pool(name="rowm", bufs=8) as rowm_pool,
        tc.tile_pool(name="stage", bufs=4) as stage_pool,
        tc.tile_pool(name="psum", bufs=8 // (gf // 512), space="PSUM") as psum_pool,
    ):
        las = inp_pool.tile([K, nt * 128], BF16, tag="la")
        nc.sync.dma_start(las[:], lhsT_a)
        ras = inp_pool.tile([K, m], BF16, tag="ra")
        nc.sync.dma_start(ras[:], rhs_a)
        lbs = inp_pool.tile([K, nt * 128], BF16, tag="lb")
        nc.sync.dma_start(lbs[:], lhsT_b)
        rbs = inp_pool.tile([K, m], BF16, tag="rb")
        nc.sync.dma_start(rbs[:], rhs_b)

        d1 = acc_pool.tile([128, nt], F32, tag="d1")
        d2 = acc_pool.tile([128, nt], F32, tag="d2")

        pools = (psum_pool, stage_pool, rowm_pool)
        for _ in range(repeat):
            _emit_layout(tc, pools, las, ras, d1, nt, m, gf)
            _emit_layout(tc, pools, lbs, rbs, d2, nt, m, gf)

        nc.sync.dma_start(d1t, d1[:])
        nc.sync.dma_start(d2t, d2[:])


def build_kernel(nc, nt=NSH // 128, m=M, gf=1024, repeat=1):
    lhsT_a = nc.dram_tensor("lhsT_a", [K, nt * 128], BF16, kind="ExternalInput")
    rhs_a = nc.dram_tensor("rhs_a", [K, m], BF16, kind="ExternalInput")
    lhsT_b = nc.dram_tensor("lhsT_b", [K, nt * 128], BF16, kind="ExternalInput")
    rhs_b = nc.dram_tensor("rhs_b", [K, m], BF16, kind="ExternalInput")
    d1t = nc.dram_tensor("d1t", [128, nt], F32, kind="ExternalOutput")
    d2t = nc.dram_tensor("d2t", [128, nt], F32, kind="ExternalOutput")
    with tile.TileContext(nc) as tc:
        build_body(tc, lhsT_a.ap(), rhs_a.ap(), lhsT_b.ap(), rhs_b.ap(),
                   d1t.ap(), d2t.ap(), nt, m, gf, repeat)
    return nc


def _split3(v):
    """v (fp32) -> three bf16 planes (as fp32) with v ~= h + l + ll."""
    bf = ml_dtypes.bfloat16
    h = v.astype(bf).astype(np.float32)
    l = (v - h).astype(bf).astype(np.float32)
    ll = (v - h - l).astype(bf).astype(np.float32)
    return h, l, ll


def _build_aug(x1, x2):
    """x1 [n,3], x2 [m,3] fp32 -> (L [24,n] bf16, R [24,m] bf16) with
    (L.T @ R)[a,b] ~= |x1_a|^2 + |x2_b|^2 - 2 x1_a.x2_b."""
    n = x1.shape[0]
    m = x2.shape[0]
    sq1 = (x1 * x1).sum(-1)
    sq2 = (x2 * x2).sum(-1)
    a = -2.0 * x1
    y = x2
    s1h, s1l, s1ll = _split3(sq1)
    s2h, s2l, s2ll = _split3(sq2)
    ah, al, all_ = _split3(a)
    yh, yl, yll = _split3(y)
    ones_n = np.ones(n, np.float32)
    ones_m = np.ones(m, np.float32)
    Ls = [s1h, s1l, s1ll, ones_n, ones_n, ones_n]
    Rs = [ones_m, ones_m, ones_m, s2h, s2l, s2ll]
    for c in range(3):
        for (L, R) in ((ah, yh), (ah, yl), (ah, yll), (al, yh), (al, yl), (all_, yh)):
            Ls.append(L[:, c])
            Rs.append(R[:, c])
    bf = ml_dtypes.bfloat16
    Lm = np.ascontiguousarray(np.stack(Ls)).astype(bf)
    Rm = np.ascontiguousarray(np.stack(Rs)).astype(bf)
    return Lm, Rm


def _make_in_maps(xyz1, xyz2):
    in_maps = []
    for c in range(NCORES):
        b, h = divmod(c, 2)
        La, Ra = _build_aug(xyz1[b, h * NSH:(h + 1) * NSH], xyz2[b])
        Lb, Rb = _build_aug(xyz2[b, h * NSH:(h + 1) * NSH], xyz1[b])
        in_maps.append({"lhsT_a": La, "rhs_a": Ra, "lhsT_b": Lb, "rhs_b": Rb})
    return in_maps


_CACHE = {}


def _get_compiled(repeat=1):
    key = ("nc", repeat)
    if key not in _CACHE:
        nc = bacc.Bacc("TRN2", target_bir_lowering=False, debug=False,
                       num_devices=NCORES)
        build_kernel(nc, repeat=repeat)
        nc.compile()
        _CACHE[key] = nc
    return _CACHE[key]


def _gather(results):
    d1 = np.empty((B, N), np.float32)
    d2 = np.empty((B, M), np.float32)
    for c in range(NCORES):
        b, h = divmod(c, 2)
        d1[b, h * NSH:(h + 1) * NSH] = results[c]["d1t"].T.reshape(-1)
        d2[b, h * NSH:(h + 1) * NSH] = results[c]["d2t"].T.reshape(-1)
    return d1, d2


def kernel(xyz1, xyz2):
    xyz1 = np.asarray(xyz1, dtype=np.float32)
    xyz2 = np.asarray(xyz2, dtype=np.float32)
    in_maps = _make_in_maps(xyz1, xyz2)
    nc = _get_compiled()
    res = bass_utils.run_bass_kernel_spmd(nc, in_maps, core_ids=list(range(NCORES)))
    return _gather(res.results)


# revision 14
# speedup vs baseline: 1195.4950x; 1195.4950x over previous
"""Chamfer distance (nn_ChamferDistance) Trainium2 Bass kernel.

Computes, for xyz1/xyz2 of shape (4, 8192, 3) fp32:
    dist[n, m] = |p_n|^2 + |q_m|^2 - 2 p_n.q_m   (per batch)
    dist1 = min over m, dist2 = min over n
Returns (dist1, dist2), each (4, 8192) fp32 — same as the reference.

Strategy:
  - The pairwise-distance matrix is produced directly by the TensorEngine via
    an augmented inner product: u_a . v_b = sq(P)[a] + sq(Q)[b] - 2 P_a.Q_b.
    All factors are split into 3 bf16 planes (hi/lo/lolo) so every product the
    PE forms is exact in fp32; dropped cross terms are ~2^-26 relative.  K=24
    contraction rows, bf16: a [128x512] distance tile costs ~512 PE cycles.
  - Sharding: 8 cores = 4 batches x 2 halves.  Each core runs TWO layouts:
      A: partitions = its half of N, free = all M  -> dist1 rows (min over free)
      B: partitions = its half of M, free = all N  -> dist2 rows (min over free)
    so both outputs are pure free-axis min-reductions; no partition reduce and
    no cross-core combine is needed.
  - Per 128-row tile, matmuls fill PSUM groups of [128, 1024] (2 banks,
    4-deep pool for overlap).  The ScalarEngine copies each group to SBUF
    (freeing the PSUM bank and taking the PSUM-port load off the DVE), then a
    single VectorEngine tensor_scalar with a min-accumulator produces the
    group's per-row min; a tiny reduce folds the group mins per tile.
"""

import numpy as np
import ml_dtypes

import concourse.bacc as bacc
import concourse.tile as tile
import concourse.mybir as mybir
from concourse import bass_utils

B = 4
N = 8192
M = 8192
NCORES = 8
NSH = N // 2          # rows per core per layout
K = 24                # augmented contraction rows

BF16 = mybir.dt.bfloat16
F32 = mybir.dt.float32
MIN = mybir.AluOpType.min
ADD = mybir.AluOpType.add
X = mybir.AxisListType.X
BIG = 1.0e30


def _emit_layout(tc, pools, lhs_sb, rhs_sb, dst, nt, m, gf):
    """One layout: dst[:, i] = min over free of (lhsT[:, i-tile].T @ rhs)."""
    nc = tc.nc
    ng = m // gf
    nj = gf // 512
    psum_pool, stage_pool, rowm_pool = pools
    for i in range(nt):
        rowm = rowm_pool.tile([128, ng], F32)
        for g in range(ng):
            ps = psum_pool.tile([128, gf], F32, tag="ps")
            for jj in range(nj):
                nc.tensor.matmul(
                    ps[:, jj * 512:(jj + 1) * 512],
                    lhs_sb[:, i * 128:(i + 1) * 128],
                    rhs_sb[:, g * gf + jj * 512: g * gf + (jj + 1) * 512],
                    start=True,
                    stop=True,
                )
            # ACT stages the PSUM group to SBUF (freeing the PSUM bank early
            # and taking the PSUM-read load off the VectorEngine); the DVE
            # then row-min-reduces it from SBUF.
            st = stage_pool.tile([128, gf], F32)
            nc.scalar.copy(st[:], ps[:])
            scr = stage_pool.tile([128, gf], F32)
            nc.vector.tensor_scalar(
                scr[:], st[:], 0.0, None, op0=ADD, op1=MIN,
                accum_out=rowm[:, g:g + 1])
        nc.vector.tensor_reduce(dst[:, i:i + 1], rowm[:], axis=X, op=MIN)


def build_body(tc, lhsT_a, rhs_a, lhsT_b, rhs_b, d1t, d2t, nt, m, gf, repeat=1):
    """Emit the kernel body into TileContext `tc`.

    lhsT_a: [K, nt*128] bf16 AP  (augmented rows of this core's N-half)
    rhs_a:  [K, m]      bf16 AP  (augmented rows of all of xyz2)
    lhsT_b: [K, nt*128] bf16 AP  (augmented rows of this core's M-half)
    rhs_b:  [K, m]      bf16 AP  (augmented rows of all of xyz1)
    d1t, d2t: [128, nt] f32 APs out (row r of tile i -> point i*128 + r)
    """
    nc = tc.nc
    with (
        tc.tile_pool(name="inp", bufs=1) as inp_pool,
        tc.tile_pool(name="acc", bufs=1) as acc_pool,
        tc.tile_pool(name="rowm", bufs=8) as rowm_pool,
        tc.tile_pool(name="stage", bufs=4) as stage_pool,
        tc.tile_pool(name="psum", bufs=8 // (gf // 512), space="PSUM") as psum_pool,
    ):
        las = inp_pool.tile([K, nt * 128], BF16, tag="la")
        nc.sync.dma_start(las[:], lhsT_a)
        ras = inp_pool.tile([K, m], BF16, tag="ra")
        nc.sync.dma_start(ras[:], rhs_a)
        lbs = inp_pool.tile([K, nt * 128], BF16, tag="lb")
        nc.sync.dma_start(lbs[:], lhsT_b)
        rbs = inp_pool.tile([K, m], BF16, tag="rb")
        nc.sync.dma_start(rbs[:], rhs_b)

        d1 = acc_pool.tile([128, nt], F32, tag="d1")
        d2 = acc_pool.tile([128, nt], F32, tag="d2")

        pools = (psum_pool, stage_pool, rowm_pool)
        for _ in range(repeat):
            _emit_layout(tc, pools, las, ras, d1, nt, m, gf)
            _emit_layout(tc, pools, lbs, rbs, d2, nt, m, gf)

        nc.sync.dma_start(d1t, d1[:])
        nc.sync.dma_start(d2t, d2[:])


def build_kernel(nc, nt=NSH // 128, m=M, gf=1024, repeat=1):
    lhsT_a = nc.dram_tensor("lhsT_a", [K, nt * 128], BF16, kind="ExternalInput")
    rhs_a = nc.dram_tensor("rhs_a", [K, m], BF16, kind="ExternalInput")
    lhsT_b = nc.dram_tensor("lhsT_b", [K, nt * 128], BF16, kind="ExternalInput")
    rhs_b = nc.dram_tensor("rhs_b", [K, m], BF16, kind="ExternalInput")
    d1t = nc.dram_tensor("d1t", [128, nt], F32, kind="ExternalOutput")
    d2t = nc.dram_tensor("d2t", [128, nt], F32, kind="ExternalOutput")
    with tile.TileContext(nc) as tc:
        build_body(tc, lhsT_a.ap(), rhs_a.ap(), lhsT_b.ap(), rhs_b.ap(),
                   d1t.ap(), d2t.ap(), nt, m, gf, repeat)
    return nc


def _split3(v):
    """v (fp32) -> three bf16 planes (as fp32) with v ~= h + l + ll."""
    bf = ml_dtypes.bfloat16
    h = v.astype(bf).astype(np.float32)
    l = (v - h).astype(bf).astype(np.float32)
    ll = (v - h - l).astype(bf).astype(np.float32)
    return h, l, ll


def _build_aug(x1, x2):
    """x1 [n,3], x2 [m,3] fp32 -> (L [24,n] bf16, R [24,m] bf16) with
    (L.T @ R)[a,b] ~= |x1_a|^2 + |x2_b|^2 - 2 x1_a.x2_b."""
    n = x1.shape[0]
    m = x2.shape[0]
    sq1 = (x1 * x1).sum(-1)
    sq2 = (x2 * x2).sum(-1)
    a = -2.0 * x1
    y = x2
    s1h, s1l, s1ll = _split3(sq1)
    s2h, s2l, s2ll = _split3(sq2)
    ah, al, all_ = _split3(a)
    yh, yl, yll = _split3(y)
    ones_n = np.ones(n, np.float32)
    ones_m = np.ones(m, np.float32)
    Ls = [s1h, s1l, s1ll, ones_n, ones_n, ones_n]
    Rs = [ones_m, ones_m, ones_m, s2h, s2l, s2ll]
    for c in range(3):
        for (L, R) in ((ah, yh), (ah, yl), (ah, yll), (al, yh), (al, yl), (all_, yh)):
            Ls.append(L[:, c])
            Rs.append(R[:, c])
    bf = ml_dtypes.bfloat16
    Lm = np.ascontiguousarray(np.stack(Ls)).astype(bf)
    Rm = np.ascontiguousarray(np.stack(Rs)).astype(bf)
    return Lm, Rm


def _make_in_maps(xyz1, xyz2):
    in_maps = []
    for c in range(NCORES):
        b, h = divmod(c, 2)
        La, Ra = _build_aug(xyz1[b, h * NSH:(h + 1) * NSH], xyz2[b])
        Lb, Rb = _build_aug(xyz2[b, h * NSH:(h + 1) * NSH], xyz1[b])
        in_maps.append({"lhsT_a": La, "rhs_a": Ra, "lhsT_b": Lb, "rhs_b": Rb})
    return in_maps


_CACHE = {}


def _get_compiled(repeat=1):
    key = ("nc", repeat)
    if key not in _CACHE:
        nc = bacc.Bacc("TRN2", target_bir_lowering=False, debug=False,
                       num_devices=NCORES)
        build_kernel(nc, repeat=repeat)
        nc.compile()
        _CACHE[key] = nc
    return _CACHE[key]


def _gather(results):
    d1 = np.empty((B, N), np.float32)
    d2 = np.empty((B, M), np.float32)
    for c in range(NCORES):
        b, h = divmod(c, 2)
        d1[b, h * NSH:(h + 1) * NSH] = results[c]["d1t"].T.reshape(-1)
        d2[b, h * NSH:(h + 1) * NSH] = results[c]["d2t"].T.reshape(-1)
    return d1, d2


def kernel(xyz1, xyz2):
    xyz1 = np.asarray(xyz1, dtype=np.float32)
    xyz2 = np.asarray(xyz2, dtype=np.float32)
    in_maps = _make_in_maps(xyz1, xyz2)
    nc = _get_compiled()
    res = bass_utils.run_bass_kernel_spmd(nc, in_maps, core_ids=list(range(NCORES)))
    return _gather(res.results)
